# revision 5
# baseline (speedup 1.0000x reference)
"""Trainium2 Bass kernel for CapsuleBlock — n-sharded across 8 cores.

Reference computation:
  hats[b,n,k,o] = sum_d x[b,n,d] * W[n,k,d,o]       x:[64,2048,8] W:[2048,16,8,16]
  3 routing iterations (softmax over k, weighted sum over n, squash over o)
  output: [64, 16, 16]

Sharding: N_in=2048 split 8 ways (256 per core), ALL 64 batches per core.
W traffic per core drops 8x vs data-parallel (2.1MB vs 16.8MB); the price
is one AllReduce of the s-partial (64KB f32) per routing iteration.
Each sweep's AR is split into two bb-halves so the first half's collective
+ squash + broadcast chain overlaps the second half's compute, and the
next sweep's first half starts without waiting for the second.

Per-core layout: n_local = g*16 + v  (g in [0,16), v in [0,16)),
b = bb*8 + bl (bb,bl in [0,8)), partition p = (bl, v) = 128.
All (k,o) free dims are k-major/o-minor: the o-tree stays bf16-packed
(stride-1 slices), the diagonal extract reduces over k via a strided
view, and every DRAM round-trip streams >=64B contiguous runs.

Einsum: per (g, bb): psum[(bl,v'),(k,o)] = LT_g_bb^T @ wt_g with
  LT[(v,d), (bl,v')] = x[b,(g,v'),d] * delta_{v,v'}  (block-diag lhsT)
  wt[(v,d), (k,o)]   = W[n=(g,v), k, d, o] bf16
s0 accumulates via 16 extra matmuls with lhsT = XT2[:, g] ([128, (bb,bl)=64])
— the v-sum happens inside the contraction, so no second full H stream.

H lives in SBUF as [p=(bl,v), f=(bb, g, k, o)] bf16 (8MB). Sweeps process
per-bb chunks: prod + o-tree on DVE (2x packed mode — all operands must be
2-byte with stride-1 last dims or the DVE silently drops to 1x; gpsimd
runs Add/Mult at 0.42 efficiency so it only gets the small bias/cch ops),
softmax (exp on Act, den/recip on DVE), LTc via per-partition-scalar
tensor_scalar (4x DVE mode), 16 accumulating s-matmuls per bb into
rotating psum, per-bb diagonal extract. The half-ARs let the tile
scheduler overlap sweep 2's first half with sweep 1's second half.

Toolchain workarounds (this container):
- walrus codegen allows only 1 sync wait/instruction -> run Bacc's
  move_matmul_waits_to_ldweights + generate_event_semaphores passes.
- constant masks precomputed on host (gpsimd int shift/mod ops broken).
- DMA source APs must keep rearranges on the DRAM side.
"""

import numpy as np

import concourse.bass as bass
import concourse.mybir as mybir
import concourse.tile as tile
from concourse.bass import ds
from concourse.bass_utils import run_bass_kernel_spmd

F32 = mybir.dt.float32
BF16 = mybir.dt.bfloat16
AX = mybir.AxisListType
OP = mybir.AluOpType
ACT_F = mybir.ActivationFunctionType

NCORES = 8
B = 64       # full batch on every core
BB = 8       # b = bb*8 + bl
BL = 8
NSH = 256    # local input capsules (2048 / 8)
K = 16       # output capsules
O = 16       # output capsule dim
D = 8        # input capsule dim
V = 16       # n's per group
G = NSH // V  # 16 groups
GL = 8       # groups per W batch
NB = G // GL  # 2 W batches
P = 128
KO = K * O   # 256

NUM_ROUTINGS = 3

# host-precomputed constant masks, packed as one [128, CF] f32 input
C_IDENT = 0              # [128, 128] identity (PE transpose)
C_M = 128                # [128, V]   M[(v,d), v'] = delta_{p>>3, v'}
C_MK = C_M + V           # [128, KO]  MKT[(bl,k'),(k,o)] = delta_{k,k'}
C_IB = C_MK + KO         # [128, BL]  Ib[(bl,v),bl'] = delta_{bl,bl'}
CF = C_IB + BL


def _build_consts() -> np.ndarray:
    c = np.zeros((P, CF), dtype=np.float32)
    p = np.arange(P)
    c[:, C_IDENT:C_IDENT + P] = np.eye(P, dtype=np.float32)
    fv = np.arange(V)
    c[:, C_M:C_M + V] = ((p >> 3)[:, None] == fv[None, :])
    # MKT: partition p=(bl,k') with k'=p%16 ; free f=(k,o) with k=f//16
    fk = np.arange(KO)
    c[:, C_MK:C_MK + KO] = ((p % K)[:, None] == (fk // O)[None, :])
    fb = np.arange(BL)
    c[:, C_IB:C_IB + BL] = ((p >> 4)[:, None] == fb[None, :])
    return c


def build_kernel(reps=1, n_sweeps=NUM_ROUTINGS - 1, no_cc=False,
                 swdge_queues=4):
    nc = bass.Bass(trn_type="TRN2", num_swdge_queues=swdge_queues,
                   num_devices=NCORES)

    x_d = nc.dram_tensor("x", [B, NSH, D], F32, kind="ExternalInput")
    w_d = nc.dram_tensor("w", [NSH, K, D, O], F32, kind="ExternalInput")
    c_d = nc.dram_tensor("consts", [P, CF], F32, kind="ExternalInput")
    out_d = nc.dram_tensor("out", [B, K, O], F32, kind="ExternalOutput")
    # scr_h[bl, k, bb4, o] bf16, one per bb-half: (k,bb,o) is contiguous so
    # the OutB broadcast read merges to 3 AP dims (DMA balancer limit); the
    # sweep's s-write lands [(bl,k'), (bb,o)] contiguously
    scr = [nc.dram_tensor(f"scr_{h}", [BL, K, BB // 2, O], BF16,
                          kind="Internal") for h in range(2)]
    # collective bounce buffers: AR0 carries s0 [(bb,bl), (k,o)] split in 2
    # bb-halves; AR1.. carry sweep s-partials [(bl,k'), (bb,o)] in 2 halves
    n_ar = n_sweeps + 1
    cc_in, cc_out = [], []
    for i in range(n_ar):
        shape = [B // 2, KO] if i == 0 else [P, BB * O // 2]
        cc_in.append([nc.dram_tensor(f"cc_in_{i}_{h}", shape, F32,
                                     kind="Internal") for h in range(2)])
        cc_out.append([nc.dram_tensor(f"cc_out_{i}_{h}", shape, F32,
                                      kind="Internal", addr_space="Shared")
                       for h in range(2)])

    with tile.TileContext(nc) as tc, nc.allow_low_precision(
            reason="bf16 capsule routing, validated vs fp32 reference"):
        for _ in range(reps):
            _capsule(tc, x_d, w_d, c_d, out_d, scr, cc_in, cc_out,
                     n_sweeps, no_cc)

    import bass_rust as _bass_rust
    _bass_rust.move_matmul_waits_to_ldweights(nc.m)
    _bass_rust.generate_event_semaphores(nc)
    return nc


def _allreduce(nc, sbuf_src_ap, dram_in, dram_out, no_cc):
    """DMA sbuf->dram_in, AllReduce (sum over 8 cores) into dram_out."""
    nc.sync.dma_start(dram_in[:], sbuf_src_ap)
    if no_cc:
        nc.sync.dma_start(dram_out[:], dram_in[:])
    else:
        nc.gpsimd.collective_compute(
            "AllReduce", OP.add,
            replica_groups=[list(range(NCORES))],
            ins=[dram_in[:]], outs=[dram_out[:]])


def _capsule(tc, x_d, w_d, c_d, out_d, scr, cc_in, cc_out, n_sweeps, no_cc):
    nc = tc.nc

    from contextlib import ExitStack
    ctx = ExitStack()
    consts = ctx.enter_context(tc.tile_pool(name="consts", bufs=1))
    hpool = ctx.enter_context(tc.tile_pool(name="hpool", bufs=1))
    small = ctx.enter_context(tc.tile_pool(name="small", bufs=2))

    # ---------------- constants (host-precomputed, one DMA) ----------------
    CON = consts.tile([P, CF], F32)
    nc.sync.dma_start(CON, c_d[:])
    ident = CON[:, ds(C_IDENT, P)]
    M = CON[:, ds(C_M, V)]
    MKT = CON[:, ds(C_MK, KO)]
    Ib = CON[:, ds(C_IB, BL)]

    H = hpool.tile([P, BB, G, KO], BF16)   # free (k,o) within KO
    bias = hpool.tile([P, BB, G, K], F32)
    nc.gpsimd.memset(bias, 0.0)   # gpsimd memset is full-rate; DVE is busy

    # ---------------- x prep: XT2[(v,d), (g, bb, bl)] bf16 ----------------
    XT2 = consts.tile([P, G, BB, BL], BF16)
    with tc.tile_pool(name="xprep", bufs=1) as xprep, \
         tc.tile_pool(name="psum_t", bufs=2, space="PSUM") as psum_t:
        X1 = xprep.tile([P, BL, V * D], F32)
        # partitions (g, bb) as separate leading dims (non-adjacent in x)
        nc.sync.dma_start(
            X1, x_d.rearrange("(bb bl) (g v) d -> g bb bl (v d)",
                              bb=BB, bl=BL, g=G, v=V))
        for bl in range(BL):
            pt = psum_t.tile([P, P], F32, tag="pt")
            nc.tensor.transpose(pt, X1[:, bl], ident)
            nc.scalar.activation(
                XT2[:, :, :, bl], pt.rearrange("p (g bb) -> p g bb", g=G),
                ACT_F.Copy)

    # ---------------- einsum ----------------
    copy_cnt = [0]

    def copy3(dst, src):
        # H psum->SBUF copies: GPSIMD cannot access PSUM, so split Act/DVE
        e = copy_cnt[0] % 2
        copy_cnt[0] += 1
        if e == 0:
            nc.scalar.activation(dst, src, ACT_F.Copy)
        else:
            nc.vector.tensor_copy(dst, src)

    with tc.tile_pool(name="ltp", bufs=1) as ltp, \
         tc.tile_pool(name="wnatp", bufs=2) as wnatp, \
         tc.tile_pool(name="wpermp", bufs=2) as wpermp, \
         tc.tile_pool(name="wtp", bufs=2) as wtp, \
         tc.tile_pool(name="psum_s", bufs=1, space="PSUM") as psum_s, \
         tc.tile_pool(name="psum_e", bufs=6, space="PSUM") as psum_e:
        # s0 accumulators, one per bb-half so the AR halves split
        ps0a = psum_s.tile([B // 2, KO], F32, name="ps0a")
        ps0b = psum_s.tile([B // 2, KO], F32, name="ps0b")
        # LT[(v,d), g, bb, (bl,v')] on DVE; per-partition scalar M[:, v']
        # rides the 4x DVE mode
        LT = ltp.tile([P, G, BB, BL, V], BF16)
        for v in range(V):
            nc.vector.tensor_scalar(
                LT[:, :, :, :, v], XT2, M[:, v:v + 1], None, op0=OP.mult)
        # hoist the full W-prep pipeline (DMA + permute + reshuffle) for both
        # batches ahead of the matmul loops — issued late, batch 1's wperm
        # would queue on Act behind the H copies and starve the PE
        wts = []
        for nb in range(NB):
            wnat = wnatp.tile([P, K * D * O], F32, tag="wnat",
                              name=f"wnat{nb}")
            nc.sync.dma_start(
                wnat, w_d[ds(nb * P, P)].rearrange("n k d o -> n (k d o)"))
            wperm = wpermp.tile([P, D, K, O], BF16, tag="wperm",
                                name=f"wperm{nb}")
            nc.scalar.activation(
                wperm, wnat.rearrange("n (k d o) -> n d k o", k=K, d=D),
                ACT_F.Copy)
            # wt reshuffle on the HWDGE (sync) queue — SWDGE would serialize
            # descriptor generation on Pool behind the H copies
            wt = wtp.tile([P, GL, KO], BF16, tag="wt", name=f"wt{nb}")
            for gi in range(GL):
                nc.sync.dma_start(
                    wt[:, gi],
                    wperm[ds(gi * V, V)].rearrange("v d k o -> v d (k o)"))
            wts.append(wt)
        for nb in range(NB):
            wt = wts[nb]
            for gi in range(GL):
                g = nb * GL + gi
                # 2-bb psum tiles (1 bank each, 6-deep rotation): finer
                # copy-release granularity keeps the PE from stalling on
                # psum-bank reuse
                for q in range(4):
                    pe = psum_e.tile([P, 2, KO], F32, tag="pe",
                                     name=f"pe_{g}_{q}")
                    for j in range(2):
                        bb = q * 2 + j
                        nc.tensor.matmul(
                            pe[:, j],
                            lhsT=LT[:, g, bb].rearrange("p bl v -> p (bl v)"),
                            rhs=wt[:, gi], start=True, stop=True)
                    copy3(H[:, ds(q * 2, 2), g], pe)
                # s0: lhsT=XT2 (no v-diag) sums over v inside the contraction
                nc.tensor.matmul(
                    ps0a, lhsT=XT2[:, g, 0:4].rearrange("p bb bl -> p (bb bl)"),
                    rhs=wt[:, gi], start=(g == 0), stop=(g == G - 1))
                nc.tensor.matmul(
                    ps0b, lhsT=XT2[:, g, 4:8].rearrange("p bb bl -> p (bb bl)"),
                    rhs=wt[:, gi], start=(g == 0), stop=(g == G - 1))

        # ---------------- s0 tail: AR per half, squash(sum/K) ----------------
        for h, ps0h in ((0, ps0a), (1, ps0b)):
            s0sb = small.tile([B // 2, KO], F32, tag=f"s0sb{h}")
            nc.scalar.activation(s0sb, ps0h, ACT_F.Copy, scale=1.0 / K)
            _allreduce(nc, s0sb, cc_in[0][h], cc_out[0][h], no_cc)
    for h in range(2):
        sred = small.tile([B // 2, KO], F32, tag=f"sred{h}")
        nc.sync.dma_start(sred, cc_out[0][h][:])
        out0 = small.tile([B // 2, K, O], BF16, tag=f"out0{h}")
        _squash_bko(nc, small, out0, sred.rearrange("b (k o) -> b k o", k=K),
                    B // 2, h)
        # scr_h[bl, k, bbq, o] <- out0[(bbq,bl), k, o]; one DMA per bbq
        # (a single DMA would need 4 unmergeable AP dims)
        for bbq in range(4):
            nc.sync.dma_start(
                scr[h][:, :, bbq].rearrange("bl k o -> bl k o"),
                out0[ds(bbq * BL, BL)])

    if n_sweeps == 0:
        # timing-diagnostic mode only (values are pre-squash, wrong layout)
        dummy = small.tile([B // 2, KO], F32, tag="sred0")
        nc.sync.dma_start(out_d[0:B // 2].rearrange("b k o -> b (k o)"), dummy)

    # ---------------- sweeps ----------------
    with tc.tile_pool(name="sweep", bufs=4) as sweep, \
         tc.tile_pool(name="prodp", bufs=4) as prodp, \
         tc.tile_pool(name="ltcp", bufs=4) as ltcp, \
         tc.tile_pool(name="psum_r", bufs=4, space="PSUM") as psum_r:
        for it in range(n_sweeps):
            last = it == n_sweeps - 1
            # OutB[p=(bl,v), (k, bbq, o)] bf16 per half, v-replicated
            # broadcast read; each half unblocks as its AR lands
            OutB = [sweep.tile([P, K, BB // 2, O], BF16, tag=f"OutB{h}",
                               name=f"OutB{h}_{it}")
                    for h in range(2)]
            for h in range(2):
                nc.scalar.dma_start(
                    OutB[h],
                    scr[h][:, None].to_broadcast((BL, V, K, BB // 2, O)))
            sAll = sweep.tile([P, BB, O], F32, tag="sAll")
            for bb in range(BB):
                # a-pass: prod + o-tree all on DVE (2x packed mode). A Pool
                # bb costs 16us and head-of-line-blocks the small bias/cch
                # ops behind it, stalling the whole softmax chain.
                eng = nc.vector
                prod = prodp.tile([P, G, K, O], BF16, tag="prod")
                eng.tensor_tensor(
                    prod, H[:, bb].rearrange("p g (k o) -> p g k o", k=K),
                    OutB[bb // 4][:, :, bb % 4][:, None].to_broadcast(
                        (P, G, K, O)),
                    op=OP.mult)
                eng.tensor_tensor(prod[:, :, :, 0:8], prod[:, :, :, 0:8],
                                  prod[:, :, :, 8:16], op=OP.add)
                eng.tensor_tensor(prod[:, :, :, 0:4], prod[:, :, :, 0:4],
                                  prod[:, :, :, 4:8], op=OP.add)
                eng.tensor_tensor(prod[:, :, :, 0:2], prod[:, :, :, 0:2],
                                  prod[:, :, :, 2:4], op=OP.add)
                ach = sweep.tile([P, G, K], BF16, tag="ach")
                eng.tensor_tensor(ach, prod[:, :, :, 0], prod[:, :, :, 1],
                                  op=OP.add)
                # bias += ach  (f32 += bf16) on Pool, off the DVE wall
                nc.gpsimd.tensor_tensor(bias[:, bb], bias[:, bb], ach,
                                        op=OP.add)
                # softmax over k
                expb = sweep.tile([P, G, K], BF16, tag="expb")
                nc.scalar.activation(expb, bias[:, bb], ACT_F.Exp)
                den = sweep.tile([P, G], F32, tag="den")
                nc.vector.tensor_reduce(den, expb, axis=AX.X, op=OP.add)
                rden = sweep.tile([P, G], F32, tag="rden")
                nc.vector.reciprocal(rden, den)
                cch = sweep.tile([P, G, K], BF16, tag="cch")
                nc.gpsimd.tensor_tensor(
                    cch, expb, rden[:, :, None].to_broadcast((P, G, K)),
                    op=OP.mult)
                # LTc[p, g, (bl',k')] = cch[p,g,k'] * Ib[p,bl'] — one
                # tensor_scalar per bl' (per-partition scalar rides 4x mode)
                LTc = ltcp.tile([P, G, BL, K], BF16, tag="LTc")
                for bl in range(BL):
                    nc.vector.tensor_scalar(
                        LTc[:, :, bl], cch, Ib[:, bl:bl + 1], None,
                        op0=OP.mult)
                pr1 = psum_r.tile([P, KO], F32, tag="pr1")
                for g in range(G):
                    nc.tensor.matmul(
                        pr1, lhsT=LTc[:, g].rearrange("p bl k -> p (bl k)"),
                        rhs=H[:, bb, g],
                        start=(g == 0), stop=(g == G - 1))
                # diagonal extract: s[(bl',k'), o] = sum_k pr1 * delta_{k,k'}
                prodD = sweep.tile([P, KO], F32, tag="prodD")
                nc.vector.tensor_tensor(prodD, pr1, MKT, op=OP.mult)
                nc.vector.tensor_reduce(
                    sAll[:, bb], prodD.rearrange("p (k o) -> p o k", k=K),
                    axis=AX.X, op=OP.add)
                if bb == 3 or bb == 7:
                    # half-AR as soon as this half's diagonals are done;
                    # the first half's collective overlaps bbs 4-7
                    h = bb // 4
                    _allreduce(nc, sAll[:, ds(h * 4, 4)].rearrange(
                        "p bb o -> p (bb o)"),
                        cc_in[it + 1][h], cc_out[it + 1][h], no_cc)
            for h in range(2):
                # boundary DMAs ride the Activation HWDGE queue — SP's queue
                # is busy issuing the cc_in/collective chain
                sred2 = sweep.tile([P, 4, O], F32, tag=f"sred2{h}")
                nc.scalar.dma_start(sred2.rearrange("p bb o -> p (bb o)"),
                                    cc_out[it + 1][h][:])
                # squash per (p=(bl,k'), bb) over o
                if last:
                    outF = sweep.tile([P, 4, O], F32, tag=f"outF{h}")
                    _squash_pbo(nc, sweep, outF, sred2, h)
                    # out_d[b,k,o]: b=(bb,bl) -> partition (bl,k), free (bb,o)
                    nc.sync.dma_start(
                        out_d.rearrange("(bb bl) k o -> (bl k) bb o", bb=BB)
                        [:, ds(h * 4, 4)], outF)
                else:
                    outN = sweep.tile([P, 4, O], BF16, tag=f"outN{h}")
                    _squash_pbo(nc, sweep, outN, sred2, h)
                    # scr_h flat layout [(bl,k'), (bbq,o)] matches outN
                    nc.scalar.dma_start(
                        scr[h].rearrange("bl k bb o -> (bl k) (bb o)"),
                        outN.rearrange("p bb o -> p (bb o)"))

    ctx.close()


def _squash_bko(nc, pool, out, s_v, nparts, tag_h):
    """squash per (b,k): norm over o only. out/s_v are [nb, K, O]."""
    sq = pool.tile([nparts, K, O], F32, tag=f"sqk_tmp{tag_h}")
    nc.vector.tensor_tensor(sq, s_v, s_v, op=OP.mult)
    ss = pool.tile([nparts, K], F32, tag=f"sqk_ss{tag_h}")
    nc.vector.tensor_reduce(ss, sq, axis=AX.X, op=OP.add)
    sc = _squash_scale(nc, pool, ss, nparts, K, f"sqk{tag_h}")
    nc.vector.tensor_tensor(
        out, s_v, sc[:, :, None].to_broadcast((nparts, K, O)), op=OP.mult)


def _squash_pbo(nc, pool, out, s_ap, tag_h):
    """squash s_ap [P=(bl,k'), nb, O] per (p, bb): norm over o."""
    nb = s_ap.shape[1]
    sq = pool.tile([P, nb, O], F32, tag=f"sqp_tmp{tag_h}")
    nc.vector.tensor_tensor(sq, s_ap, s_ap, op=OP.mult)
    ss = pool.tile([P, nb], F32, tag=f"sqp_ss{tag_h}")
    nc.vector.tensor_reduce(ss, sq, axis=AX.X, op=OP.add)
    sc = _squash_scale(nc, pool, ss, P, nb, f"sqp{tag_h}")
    nc.vector.tensor_tensor(
        out, s_ap, sc[:, :, None].to_broadcast((P, nb, O)), op=OP.mult)


def _squash_scale(nc, pool, ss, np_, nf, tag):
    """sc = sqrt(ss)/(1+ss) elementwise on [np_, nf]."""
    rt = pool.tile([np_, nf], F32, tag=f"{tag}_rt")
    nc.scalar.activation(rt, ss, ACT_F.Sqrt)
    dn = pool.tile([np_, nf], F32, tag=f"{tag}_dn")
    nc.vector.tensor_scalar(dn, ss, 1.0, None, op0=OP.add)
    rc = pool.tile([np_, nf], F32, tag=f"{tag}_rc")
    nc.vector.reciprocal(rc, dn)
    sc = pool.tile([np_, nf], F32, tag=f"{tag}_sc")
    nc.vector.tensor_tensor(sc, rt, rc, op=OP.mult)
    return sc


_NC_CACHE = None


def make_in_maps(x: np.ndarray, W: np.ndarray) -> list:
    consts = _build_consts()
    return [{"x": np.ascontiguousarray(x[:, c * NSH:(c + 1) * NSH]),
             "w": np.ascontiguousarray(W[c * NSH:(c + 1) * NSH]),
             "consts": consts}
            for c in range(NCORES)]


def kernel(x: np.ndarray, W: np.ndarray) -> np.ndarray:
    global _NC_CACHE
    x = np.ascontiguousarray(x, dtype=np.float32)
    W = np.ascontiguousarray(W, dtype=np.float32)
    if _NC_CACHE is None:
        _NC_CACHE = build_kernel()
    nc = _NC_CACHE
    consts = _build_consts()
    in_maps = [{"x": np.ascontiguousarray(x[:, c * NSH:(c + 1) * NSH]),
                "w": np.ascontiguousarray(W[c * NSH:(c + 1) * NSH]),
                "consts": consts}
               for c in range(NCORES)]
    res = run_bass_kernel_spmd(nc, in_maps, core_ids=list(range(NCORES)))
    return res.results[0]["out"]


# revision 7
# speedup vs baseline: 1.0718x; 1.0718x over previous
"""Trainium2 Bass kernel for CapsuleBlock — n-sharded across 8 cores.

Reference computation:
  hats[b,n,k,o] = sum_d x[b,n,d] * W[n,k,d,o]       x:[64,2048,8] W:[2048,16,8,16]
  3 routing iterations (softmax over k, weighted sum over n, squash over o)
  output: [64, 16, 16]

Sharding: N_in=2048 split 8 ways (256 per core), ALL 64 batches per core.
W traffic per core drops 8x vs data-parallel (2.1MB vs 16.8MB); the price
is one AllReduce of the s-partial (64KB f32) per routing iteration.
Each sweep's AR is split into two bb-halves so the first half's collective
+ squash + broadcast chain overlaps the second half's compute, and the
next sweep's first half starts without waiting for the second.

Per-core layout: n_local = g*16 + v  (g in [0,16), v in [0,16)),
b = bb*8 + bl (bb,bl in [0,8)), partition p = (bl, v) = 128.
All (k,o) free dims are k-major/o-minor: the o-tree stays bf16-packed
(stride-1 slices), the diagonal extract reduces over k via a strided
view, and every DRAM round-trip streams >=64B contiguous runs.

Einsum: per (g, bb): psum[(bl,v'),(k,o)] = LT_g_bb^T @ wt_g with
  LT[(v,d), (bl,v')] = x[b,(g,v'),d] * delta_{v,v'}  (block-diag lhsT)
  wt[(v,d), (k,o)]   = W[n=(g,v), k, d, o] bf16
s0 accumulates via 16 extra matmuls with lhsT = XT2[:, g] ([128, (bb,bl)=64])
— the v-sum happens inside the contraction, so no second full H stream.

H lives in SBUF as [p=(bl,v), f=(bb, g, k, o)] bf16 (8MB). Sweeps process
per-bb chunks: prod + o-tree on DVE (2x packed mode — all operands must be
2-byte with stride-1 last dims or the DVE silently drops to 1x; gpsimd
runs Add/Mult at 0.42 efficiency so it only gets the small bias/cch ops),
softmax (exp on Act, den/recip on DVE), LTc via per-partition-scalar
tensor_scalar (4x DVE mode), 16 accumulating s-matmuls per bb into
rotating psum, per-bb diagonal extract. The half-ARs let the tile
scheduler overlap sweep 2's first half with sweep 1's second half.

Toolchain workarounds (this container):
- walrus codegen allows only 1 sync wait/instruction -> run Bacc's
  move_matmul_waits_to_ldweights + generate_event_semaphores passes.
- constant masks precomputed on host (gpsimd int shift/mod ops broken).
- DMA source APs must keep rearranges on the DRAM side.
"""

import numpy as np

import concourse.bass as bass
import concourse.mybir as mybir
import concourse.tile as tile
from concourse.bass import ds
from concourse.bass_utils import run_bass_kernel_spmd

F32 = mybir.dt.float32
BF16 = mybir.dt.bfloat16
AX = mybir.AxisListType
OP = mybir.AluOpType
ACT_F = mybir.ActivationFunctionType

NCORES = 8
B = 64       # full batch on every core
BB = 8       # b = bb*8 + bl
BL = 8
NSH = 256    # local input capsules (2048 / 8)
K = 16       # output capsules
O = 16       # output capsule dim
D = 8        # input capsule dim
V = 16       # n's per group
G = NSH // V  # 16 groups
GL = 8       # groups per W batch
NB = G // GL  # 2 W batches
P = 128
KO = K * O   # 256

NUM_ROUTINGS = 3

# host-precomputed constant masks, packed as one [128, CF] f32 input
C_IDENT = 0              # [128, 128] identity (PE transpose)
C_M = 128                # [128, V]   M[(v,d), v'] = delta_{p>>3, v'}
C_MK = C_M + V           # [128, KO]  MKT[(bl,k'),(k,o)] = delta_{k,k'}
C_IB = C_MK + KO         # [128, BL]  Ib[(bl,v),bl'] = delta_{bl,bl'}
CF = C_IB + BL


def _build_consts() -> np.ndarray:
    c = np.zeros((P, CF), dtype=np.float32)
    p = np.arange(P)
    c[:, C_IDENT:C_IDENT + P] = np.eye(P, dtype=np.float32)
    fv = np.arange(V)
    c[:, C_M:C_M + V] = ((p >> 3)[:, None] == fv[None, :])
    # MKT: partition p=(bl,k') with k'=p%16 ; free f=(k,o) with k=f//16
    fk = np.arange(KO)
    c[:, C_MK:C_MK + KO] = ((p % K)[:, None] == (fk // O)[None, :])
    fb = np.arange(BL)
    c[:, C_IB:C_IB + BL] = ((p >> 4)[:, None] == fb[None, :])
    return c


def build_kernel(reps=1, n_sweeps=NUM_ROUTINGS - 1, no_cc=False,
                 swdge_queues=4):
    nc = bass.Bass(trn_type="TRN2", num_swdge_queues=swdge_queues,
                   num_devices=NCORES)

    x_d = nc.dram_tensor("x", [B, NSH, D], F32, kind="ExternalInput")
    w_d = nc.dram_tensor("w", [NSH, K, D, O], F32, kind="ExternalInput")
    c_d = nc.dram_tensor("consts", [P, CF], F32, kind="ExternalInput")
    out_d = nc.dram_tensor("out", [B, K, O], F32, kind="ExternalOutput")
    # scr_h[bl, k, bb4, o] bf16, one per bb-half: (k,bb,o) is contiguous so
    # the OutB broadcast read merges to 3 AP dims (DMA balancer limit); the
    # sweep's s-write lands [(bl,k'), (bb,o)] contiguously
    scr = [nc.dram_tensor(f"scr_{h}", [BL, K, BB // 2, O], BF16,
                          kind="Internal") for h in range(2)]
    # collective bounce buffers: AR0 carries s0 [(bb,bl), (k,o)] split in 2
    # bb-halves; AR1.. carry sweep s-partials [(bl,k'), (bb,o)] in 2 halves
    n_ar = n_sweeps + 1
    cc_in, cc_out = [], []
    for i in range(n_ar):
        shape = [B // 2, KO] if i == 0 else [P, BB * O // 2]
        cc_in.append([nc.dram_tensor(f"cc_in_{i}_{h}", shape, F32,
                                     kind="Internal") for h in range(2)])
        cc_out.append([nc.dram_tensor(f"cc_out_{i}_{h}", shape, F32,
                                      kind="Internal", addr_space="Shared")
                       for h in range(2)])

    with tile.TileContext(nc) as tc, nc.allow_low_precision(
            reason="bf16 capsule routing, validated vs fp32 reference"):
        for _ in range(reps):
            _capsule(tc, x_d, w_d, c_d, out_d, scr, cc_in, cc_out,
                     n_sweeps, no_cc)

    import bass_rust as _bass_rust
    _bass_rust.move_matmul_waits_to_ldweights(nc.m)
    _bass_rust.generate_event_semaphores(nc)
    return nc


def _allreduce(nc, sbuf_src_ap, dram_in, dram_out, no_cc):
    """DMA sbuf->dram_in, AllReduce (sum over 8 cores) into dram_out."""
    nc.sync.dma_start(dram_in[:], sbuf_src_ap)
    if no_cc:
        nc.sync.dma_start(dram_out[:], dram_in[:])
    else:
        nc.gpsimd.collective_compute(
            "AllReduce", OP.add,
            replica_groups=[list(range(NCORES))],
            ins=[dram_in[:]], outs=[dram_out[:]])


def _capsule(tc, x_d, w_d, c_d, out_d, scr, cc_in, cc_out, n_sweeps, no_cc):
    nc = tc.nc

    from contextlib import ExitStack
    ctx = ExitStack()
    consts = ctx.enter_context(tc.tile_pool(name="consts", bufs=1))
    hpool = ctx.enter_context(tc.tile_pool(name="hpool", bufs=1))
    small = ctx.enter_context(tc.tile_pool(name="small", bufs=2))

    # ---------------- constants (host-precomputed, one DMA) ----------------
    CON = consts.tile([P, CF], F32)
    nc.sync.dma_start(CON, c_d[:])
    ident = CON[:, ds(C_IDENT, P)]
    M = CON[:, ds(C_M, V)]
    MKT = CON[:, ds(C_MK, KO)]
    Ib = CON[:, ds(C_IB, BL)]

    H = hpool.tile([P, BB, G, KO], BF16)   # free (k,o) within KO
    bias = hpool.tile([P, BB, G, K], F32)
    nc.gpsimd.memset(bias, 0.0)   # gpsimd memset is full-rate; DVE is busy

    # ---------------- x prep: XT2[(v,d), (g, bb, bl)] bf16 ----------------
    XT2 = consts.tile([P, G, BB, BL], BF16)
    with tc.tile_pool(name="xprep", bufs=1) as xprep, \
         tc.tile_pool(name="psum_t", bufs=2, space="PSUM") as psum_t:
        X1 = xprep.tile([P, BL, V * D], F32)
        # partitions (g, bb) as separate leading dims (non-adjacent in x)
        nc.sync.dma_start(
            X1, x_d.rearrange("(bb bl) (g v) d -> g bb bl (v d)",
                              bb=BB, bl=BL, g=G, v=V))
        for bl in range(BL):
            pt = psum_t.tile([P, P], F32, tag="pt")
            nc.tensor.transpose(pt, X1[:, bl], ident)
            nc.scalar.activation(
                XT2[:, :, :, bl], pt.rearrange("p (g bb) -> p g bb", g=G),
                ACT_F.Copy)

    # ---------------- einsum ----------------
    copy_cnt = [0]

    def copy3(dst, src):
        # H psum->SBUF copies: GPSIMD cannot access PSUM, so split Act/DVE
        e = copy_cnt[0] % 2
        copy_cnt[0] += 1
        if e == 0:
            nc.scalar.activation(dst, src, ACT_F.Copy)
        else:
            nc.vector.tensor_copy(dst, src)

    with tc.tile_pool(name="ltp", bufs=1) as ltp, \
         tc.tile_pool(name="wnatp", bufs=2) as wnatp, \
         tc.tile_pool(name="wpermp", bufs=2) as wpermp, \
         tc.tile_pool(name="wtp", bufs=2) as wtp, \
         tc.tile_pool(name="psum_s", bufs=1, space="PSUM") as psum_s, \
         tc.tile_pool(name="psum_e", bufs=6, space="PSUM") as psum_e:
        # s0 accumulators, one per bb-half so the AR halves split
        ps0a = psum_s.tile([B // 2, KO], F32, name="ps0a")
        ps0b = psum_s.tile([B // 2, KO], F32, name="ps0b")
        # LT[(v,d), g, bb, (bl,v')] on DVE; per-partition scalar M[:, v']
        # rides the 4x DVE mode
        LT = ltp.tile([P, G, BB, BL, V], BF16)
        for v in range(V):
            nc.vector.tensor_scalar(
                LT[:, :, :, :, v], XT2, M[:, v:v + 1], None, op0=OP.mult)
        # hoist the full W-prep pipeline (DMA + permute + reshuffle) for both
        # batches ahead of the matmul loops — issued late, batch 1's wperm
        # would queue on Act behind the H copies and starve the PE
        wts = []
        for nb in range(NB):
            wnat = wnatp.tile([P, K * D * O], F32, tag="wnat",
                              name=f"wnat{nb}")
            nc.sync.dma_start(
                wnat, w_d[ds(nb * P, P)].rearrange("n k d o -> n (k d o)"))
            wperm = wpermp.tile([P, D, K, O], BF16, tag="wperm",
                                name=f"wperm{nb}")
            nc.scalar.activation(
                wperm, wnat.rearrange("n (k d o) -> n d k o", k=K, d=D),
                ACT_F.Copy)
            # wt reshuffle on the HWDGE (sync) queue — SWDGE would serialize
            # descriptor generation on Pool behind the H copies
            wt = wtp.tile([P, GL, KO], BF16, tag="wt", name=f"wt{nb}")
            for gi in range(GL):
                nc.sync.dma_start(
                    wt[:, gi],
                    wperm[ds(gi * V, V)].rearrange("v d k o -> v d (k o)"))
            wts.append(wt)
        for nb in range(NB):
            wt = wts[nb]
            for gi in range(GL):
                g = nb * GL + gi
                # 2-bb psum tiles (1 bank each, 6-deep rotation): finer
                # copy-release granularity keeps the PE from stalling on
                # psum-bank reuse
                for q in range(4):
                    pe = psum_e.tile([P, 2, KO], F32, tag="pe",
                                     name=f"pe_{g}_{q}")
                    for j in range(2):
                        bb = q * 2 + j
                        nc.tensor.matmul(
                            pe[:, j],
                            lhsT=LT[:, g, bb].rearrange("p bl v -> p (bl v)"),
                            rhs=wt[:, gi], start=True, stop=True)
                    copy3(H[:, ds(q * 2, 2), g], pe)
                # s0: lhsT=XT2 (no v-diag) sums over v inside the contraction
                nc.tensor.matmul(
                    ps0a, lhsT=XT2[:, g, 0:4].rearrange("p bb bl -> p (bb bl)"),
                    rhs=wt[:, gi], start=(g == 0), stop=(g == G - 1))
                nc.tensor.matmul(
                    ps0b, lhsT=XT2[:, g, 4:8].rearrange("p bb bl -> p (bb bl)"),
                    rhs=wt[:, gi], start=(g == 0), stop=(g == G - 1))

        # ---------------- s0 tail: AR per half, squash(sum/K) ----------------
        for h, ps0h in ((0, ps0a), (1, ps0b)):
            s0sb = small.tile([B // 2, KO], F32, tag=f"s0sb{h}")
            nc.scalar.activation(s0sb, ps0h, ACT_F.Copy, scale=1.0 / K)
            _allreduce(nc, s0sb, cc_in[0][h], cc_out[0][h], no_cc)
    for h in range(2):
        sred = small.tile([B // 2, KO], F32, tag=f"sred{h}")
        nc.sync.dma_start(sred, cc_out[0][h][:])
        out0 = small.tile([B // 2, K, O], BF16, tag=f"out0{h}")
        _squash_bko(nc, small, out0, sred.rearrange("b (k o) -> b k o", k=K),
                    B // 2, h)
        # scr_h[bl, k, bbq, o] <- out0[(bbq,bl), k, o]; one DMA per bbq
        # (a single DMA would need 4 unmergeable AP dims)
        for bbq in range(4):
            nc.sync.dma_start(
                scr[h][:, :, bbq].rearrange("bl k o -> bl k o"),
                out0[ds(bbq * BL, BL)])

    if n_sweeps == 0:
        # timing-diagnostic mode only (values are pre-squash, wrong layout)
        dummy = small.tile([B // 2, KO], F32, tag="sred0")
        nc.sync.dma_start(out_d[0:B // 2].rearrange("b k o -> b (k o)"), dummy)

    # ---------------- sweeps ----------------
    with tc.tile_pool(name="sweep", bufs=4) as sweep, \
         tc.tile_pool(name="prodp", bufs=4) as prodp, \
         tc.tile_pool(name="ltcp", bufs=4) as ltcp, \
         tc.tile_pool(name="psum_r", bufs=4, space="PSUM") as psum_r:
        for it in range(n_sweeps):
            last = it == n_sweeps - 1
            # OutB[p=(bl,v), (k, bbq, o)] bf16 per half, v-replicated
            # broadcast read; each half unblocks as its AR lands
            OutB = [sweep.tile([P, K, BB // 2, O], BF16, tag=f"OutB{h}",
                               name=f"OutB{h}_{it}")
                    for h in range(2)]
            for h in range(2):
                nc.scalar.dma_start(
                    OutB[h],
                    scr[h][:, None].to_broadcast((BL, V, K, BB // 2, O)))
            sAll = sweep.tile([P, BB, O], F32, tag="sAll")
            for bb in range(BB):
                # a-pass: prod + o-tree all on DVE (2x packed mode). A Pool
                # bb costs 16us and head-of-line-blocks the small bias/cch
                # ops behind it, stalling the whole softmax chain.
                # high_priority hoists the a-pass ~1 bb earlier in the
                # scheduler's perceived issue order so DVE doesn't park
                # in-order behind the previous bb's den reduce (whose exp
                # input lags on the Act/Pool chain) while this prod is
                # data-ready.
                eng = nc.vector
                with tc.high_priority(offset=40):
                    prod = prodp.tile([P, G, K, O], BF16, tag="prod")
                    eng.tensor_tensor(
                        prod, H[:, bb].rearrange("p g (k o) -> p g k o", k=K),
                        OutB[bb // 4][:, :, bb % 4][:, None].to_broadcast(
                            (P, G, K, O)),
                        op=OP.mult)
                    eng.tensor_tensor(prod[:, :, :, 0:8], prod[:, :, :, 0:8],
                                      prod[:, :, :, 8:16], op=OP.add)
                    eng.tensor_tensor(prod[:, :, :, 0:4], prod[:, :, :, 0:4],
                                      prod[:, :, :, 4:8], op=OP.add)
                    eng.tensor_tensor(prod[:, :, :, 0:2], prod[:, :, :, 0:2],
                                      prod[:, :, :, 2:4], op=OP.add)
                    ach = sweep.tile([P, G, K], BF16, tag="ach")
                    eng.tensor_tensor(ach, prod[:, :, :, 0],
                                      prod[:, :, :, 1], op=OP.add)
                # bias += ach  (f32 += bf16) on Pool, off the DVE wall
                nc.gpsimd.tensor_tensor(bias[:, bb], bias[:, bb], ach,
                                        op=OP.add)
                # softmax over k
                expb = sweep.tile([P, G, K], BF16, tag="expb")
                nc.scalar.activation(expb, bias[:, bb], ACT_F.Exp)
                den = sweep.tile([P, G], F32, tag="den")
                nc.vector.tensor_reduce(den, expb, axis=AX.X, op=OP.add)
                rden = sweep.tile([P, G], F32, tag="rden")
                nc.vector.reciprocal(rden, den)
                cch = sweep.tile([P, G, K], BF16, tag="cch")
                nc.gpsimd.tensor_tensor(
                    cch, expb, rden[:, :, None].to_broadcast((P, G, K)),
                    op=OP.mult)
                # LTc[p, g, (bl',k')] = cch[p,g,k'] * Ib[p,bl'] — one
                # tensor_scalar per bl' (per-partition scalar rides 4x mode)
                LTc = ltcp.tile([P, G, BL, K], BF16, tag="LTc")
                for bl in range(BL):
                    nc.vector.tensor_scalar(
                        LTc[:, :, bl], cch, Ib[:, bl:bl + 1], None,
                        op0=OP.mult)
                pr1 = psum_r.tile([P, KO], F32, tag="pr1")
                for g in range(G):
                    nc.tensor.matmul(
                        pr1, lhsT=LTc[:, g].rearrange("p bl k -> p (bl k)"),
                        rhs=H[:, bb, g],
                        start=(g == 0), stop=(g == G - 1))
                # diagonal extract: s[(bl',k'), o] = sum_k pr1 * delta_{k,k'}
                # pr1 leaves psum via Act (Pool can't read PSUM), the mask
                # mult runs on Pool — only the reduce stays on the DVE wall
                pr1s = sweep.tile([P, KO], F32, tag="pr1s")
                nc.scalar.activation(pr1s, pr1, ACT_F.Copy)
                prodD = sweep.tile([P, KO], F32, tag="prodD")
                nc.gpsimd.tensor_tensor(prodD, pr1s, MKT, op=OP.mult)
                nc.vector.tensor_reduce(
                    sAll[:, bb], prodD.rearrange("p (k o) -> p o k", k=K),
                    axis=AX.X, op=OP.add)
                if bb == 3 or bb == 7:
                    # half-AR as soon as this half's diagonals are done;
                    # the first half's collective overlaps bbs 4-7
                    h = bb // 4
                    _allreduce(nc, sAll[:, ds(h * 4, 4)].rearrange(
                        "p bb o -> p (bb o)"),
                        cc_in[it + 1][h], cc_out[it + 1][h], no_cc)
            for h in range(2):
                # boundary DMAs ride the Activation HWDGE queue — SP's queue
                # is busy issuing the cc_in/collective chain
                sred2 = sweep.tile([P, 4, O], F32, tag=f"sred2{h}")
                nc.scalar.dma_start(sred2.rearrange("p bb o -> p (bb o)"),
                                    cc_out[it + 1][h][:])
                # squash per (p=(bl,k'), bb) over o
                if last:
                    outF = sweep.tile([P, 4, O], F32, tag=f"outF{h}")
                    _squash_pbo(nc, sweep, outF, sred2, h)
                    # out_d[b,k,o]: b=(bb,bl) -> partition (bl,k), free (bb,o)
                    nc.sync.dma_start(
                        out_d.rearrange("(bb bl) k o -> (bl k) bb o", bb=BB)
                        [:, ds(h * 4, 4)], outF)
                else:
                    outN = sweep.tile([P, 4, O], BF16, tag=f"outN{h}")
                    _squash_pbo(nc, sweep, outN, sred2, h)
                    # scr_h flat layout [(bl,k'), (bbq,o)] matches outN
                    nc.scalar.dma_start(
                        scr[h].rearrange("bl k bb o -> (bl k) (bb o)"),
                        outN.rearrange("p bb o -> p (bb o)"))

    ctx.close()


def _squash_bko(nc, pool, out, s_v, nparts, tag_h):
    """squash per (b,k): norm over o only. out/s_v are [nb, K, O]."""
    sq = pool.tile([nparts, K, O], F32, tag=f"sqk_tmp{tag_h}")
    nc.vector.tensor_tensor(sq, s_v, s_v, op=OP.mult)
    ss = pool.tile([nparts, K], F32, tag=f"sqk_ss{tag_h}")
    nc.vector.tensor_reduce(ss, sq, axis=AX.X, op=OP.add)
    sc = _squash_scale(nc, pool, ss, nparts, K, f"sqk{tag_h}")
    nc.vector.tensor_tensor(
        out, s_v, sc[:, :, None].to_broadcast((nparts, K, O)), op=OP.mult)


def _squash_pbo(nc, pool, out, s_ap, tag_h):
    """squash s_ap [P=(bl,k'), nb, O] per (p, bb): norm over o."""
    nb = s_ap.shape[1]
    sq = pool.tile([P, nb, O], F32, tag=f"sqp_tmp{tag_h}")
    nc.vector.tensor_tensor(sq, s_ap, s_ap, op=OP.mult)
    ss = pool.tile([P, nb], F32, tag=f"sqp_ss{tag_h}")
    nc.vector.tensor_reduce(ss, sq, axis=AX.X, op=OP.add)
    sc = _squash_scale(nc, pool, ss, P, nb, f"sqp{tag_h}")
    nc.vector.tensor_tensor(
        out, s_ap, sc[:, :, None].to_broadcast((P, nb, O)), op=OP.mult)


def _squash_scale(nc, pool, ss, np_, nf, tag):
    """sc = sqrt(ss)/(1+ss) elementwise on [np_, nf]."""
    rt = pool.tile([np_, nf], F32, tag=f"{tag}_rt")
    nc.scalar.activation(rt, ss, ACT_F.Sqrt)
    dn = pool.tile([np_, nf], F32, tag=f"{tag}_dn")
    nc.vector.tensor_scalar(dn, ss, 1.0, None, op0=OP.add)
    rc = pool.tile([np_, nf], F32, tag=f"{tag}_rc")
    nc.vector.reciprocal(rc, dn)
    sc = pool.tile([np_, nf], F32, tag=f"{tag}_sc")
    nc.vector.tensor_tensor(sc, rt, rc, op=OP.mult)
    return sc


_NC_CACHE = None


def make_in_maps(x: np.ndarray, W: np.ndarray) -> list:
    consts = _build_consts()
    return [{"x": np.ascontiguousarray(x[:, c * NSH:(c + 1) * NSH]),
             "w": np.ascontiguousarray(W[c * NSH:(c + 1) * NSH]),
             "consts": consts}
            for c in range(NCORES)]


def kernel(x: np.ndarray, W: np.ndarray) -> np.ndarray:
    global _NC_CACHE
    x = np.ascontiguousarray(x, dtype=np.float32)
    W = np.ascontiguousarray(W, dtype=np.float32)
    if _NC_CACHE is None:
        _NC_CACHE = build_kernel()
    nc = _NC_CACHE
    consts = _build_consts()
    in_maps = [{"x": np.ascontiguousarray(x[:, c * NSH:(c + 1) * NSH]),
                "w": np.ascontiguousarray(W[c * NSH:(c + 1) * NSH]),
                "consts": consts}
               for c in range(NCORES)]
    res = run_bass_kernel_spmd(nc, in_maps, core_ids=list(range(NCORES)))
    return res.results[0]["out"]
